# revision 1
# baseline (speedup 1.0000x reference)
"""Trainium2 Bass kernel for nn_DeepNet (dense MLP with BatchNorm over batch).

Reference computation (N=8192 rows, K=2 in/out features, H=4096 hidden, D=3):
    X = relu(X_in @ W_first + b_first)                      # [N, H]
    for i in range(3):
        Xh = relu(X @ W_h[i] + b_h[i])                      # [N, H]
        sq = rowwise_sum(Xh * Xh)                           # [N, 1]
        X  = bn(sq + Xh + X)        # batch stats over N, per hidden unit
    out = bn(X @ W_last + b_last + X_in)                    # [N, 2]

Strategy: data-parallel over N across 8 NeuronCores (1024 rows/core).
Activations live in SBUF in *transposed* layout A[h, m] = X[m, h] so that
  - hidden matmuls use weights as the stationary operand in natural [K, N]
    layout (no transposes anywhere),
  - BatchNorm's per-hidden-unit batch stats are free-axis reductions.
Matmuls run as float32r (full fp32 storage; PE truncates to ~13-bit mantissa,
4x faster than true fp32; end-to-end error ~4e-5).
BatchNorm uses a Welford-style merge: each core computes its local per-unit
(mean, mean^2, centered-M2); one 48KB AllReduce per BN layer combines them:
  var = sum_s M2_s / N + mean_s(mu_s^2) - mu^2   (8 equal shards of 1024)
which avoids the catastrophic E[x^2]-mu^2 cancellation (values ~2048, std ~50).
"""

import numpy as np

N_CORES = 8
N = 8192
NLOC = N // N_CORES  # 1024 rows per core
KIO = 2
H = 4096
HT = H // 128  # 32 hidden-dim tiles
D = 3
MC = 512  # matmul moving-operand chunk (fp32 max)
EPS = 1e-5

_CACHE = {}


def _build():
    import concourse.bass as bass  # noqa: F401  (registers engines)
    import concourse.mybir as mybir
    import concourse.tile as tile
    from concourse import bacc

    F32 = mybir.dt.float32
    F32R = mybir.dt.float32r
    BF16 = mybir.dt.bfloat16
    ALU = mybir.AluOpType
    ACTF = mybir.ActivationFunctionType
    AX = mybir.AxisListType.X

    nc = bacc.Bacc("TRN2", target_bir_lowering=False, debug=False, num_devices=N_CORES)

    xin = nc.dram_tensor("x_in", [NLOC, KIO], F32R, kind="ExternalInput")
    wf = nc.dram_tensor("w_first", [KIO, H], F32R, kind="ExternalInput")
    bf = nc.dram_tensor("b_first", [H], F32, kind="ExternalInput")
    wh = nc.dram_tensor("w_h", [D, H, H], F32R, kind="ExternalInput")
    bh = nc.dram_tensor("b_h", [D, H], F32, kind="ExternalInput")
    wl = nc.dram_tensor("w_last", [H, KIO], F32R, kind="ExternalInput")
    bl = nc.dram_tensor("b_last", [KIO], F32, kind="ExternalInput")
    yx = nc.dram_tensor("y", [NLOC, KIO], F32, kind="ExternalOutput")
    ah_spill = nc.dram_tensor("ah_spill", [H, NLOC], BF16)

    groups = [list(range(N_CORES))]

    def mm_cast(ap):
        return ap

    with tile.TileContext(nc) as tc:
        with (
            tc.tile_pool(name="a", bufs=1) as apool,
            tc.tile_pool(name="w", bufs=2) as wpool,
            tc.tile_pool(name="ahw", bufs=2) as ahw_pool,
            tc.tile_pool(name="ypw", bufs=2) as ypw_pool,
            tc.tile_pool(name="ahr", bufs=3) as ahr_pool,
            tc.tile_pool(name="sc", bufs=2) as sc_pool,
            tc.tile_pool(name="sqw", bufs=1) as sqw_pool,
            tc.tile_pool(name="st", bufs=1) as st_pool,
            tc.tile_pool(name="st2", bufs=2) as st2_pool,
            tc.tile_pool(name="ps", bufs=2, space="PSUM") as ps_pool,
            tc.tile_pool(name="sqps", bufs=1, space="PSUM") as sq_pool,
            tc.tile_pool(name="dram", bufs=1, space="DRAM") as dpool,
        ):
            A = apool.tile([128, HT * NLOC], F32R)

            ones_f = st_pool.tile([128, 1], F32)
            nc.vector.memset(ones_f[:], 1.0)
            ones_t = st_pool.tile([128, 1], F32R)
            nc.vector.tensor_copy(ones_t[:], ones_f[:])
            eps_t = st_pool.tile([128, 1], F32)
            nc.vector.memset(eps_t[:], EPS)
            xtr = st_pool.tile([KIO, NLOC], F32R)
            nc.sync.dma_start(xtr[:], xin.rearrange("m k -> k m"))
            bf_t = st_pool.tile([128, HT], F32)
            nc.sync.dma_start(bf_t[:], bf.rearrange("(t p) -> p t", p=128))

            suma_a = st_pool.tile([128, HT], F32)
            suma_b = st_pool.tile([128, HT], F32)
            sumA = st_pool.tile([128, HT], F32)
            m2a = st_pool.tile([128, HT], F32)
            mu_s = st_pool.tile([128, HT], F32)
            mu2 = st_pool.tile([128, HT], F32)
            tmp1 = st_pool.tile([128, HT], F32)
            tmp2 = st_pool.tile([128, HT], F32)
            var_t = st_pool.tile([128, HT], F32)
            std_t = st_pool.tile([128, HT], F32)
            scale_t = st_pool.tile([128, HT], F32)
            ds_t = st_pool.tile([128, HT], F32)
            sq_sb = st_pool.tile([1, NLOC], F32)
            ssq_bc = st_pool.tile([128, 1], F32)
            bsq = st_pool.tile([128, NLOC], F32)

            # ---------------- first layer: A = relu(W_first^T X_in^T + b) ----
            wf_t = wpool.tile([KIO, H], F32R, tag="w")
            nc.sync.dma_start(wf_t[:], wf[:, :])
            for n in range(HT):
                ps0 = ps_pool.tile([128, MC], F32, tag="ps0")
                ps1 = ps_pool.tile([128, MC], F32, tag="ps1")
                lhsT = mm_cast(wf_t[:, n * 128 : (n + 1) * 128])
                nc.tensor.matmul(ps0[:], lhsT, mm_cast(xtr[:, 0:MC]), start=True, stop=True)
                nc.tensor.matmul(ps1[:], lhsT, mm_cast(xtr[:, MC:NLOC]), start=True, stop=True)
                a_sl = A[:, n * NLOC : (n + 1) * NLOC]
                nc.scalar.activation(
                    a_sl[:, 0:MC], ps0[:], ACTF.Relu,
                    bias=bf_t[:, n : n + 1], accum_out=suma_a[:, n : n + 1],
                )
                nc.scalar.activation(
                    a_sl[:, MC:NLOC], ps1[:], ACTF.Relu,
                    bias=bf_t[:, n : n + 1], accum_out=suma_b[:, n : n + 1],
                )
            nc.vector.tensor_tensor(sumA[:], suma_a[:], suma_b[:], op=ALU.add)

            # warm up the collective rings while the PE is busy with layer 1:
            # the first AllReduce otherwise pays ~20us of cold-start inside the
            # first BN tail
            ccw_in = dpool.tile([1, 1], F32, tag="ccw_in")
            ccw_out = dpool.tile([1, 1], F32, tag="ccw_out")
            nc.gpsimd.dma_start(ccw_in[:], ones_f[0:1, 0:1])
            nc.gpsimd.collective_compute(
                "AllReduce", ALU.add, replica_groups=groups,
                ins=[ccw_in.opt()], outs=[ccw_out.opt()],
            )

            # ---------------- hidden layers ----------------
            for li in range(D):
                bh_t = st2_pool.tile([128, HT], F32, tag="bh")
                nc.sync.dma_start(bh_t[:], bh[li].rearrange("(t p) -> p t", p=128))
                sqp0 = sq_pool.tile([1, MC], F32, tag="sq0")
                sqp1 = sq_pool.tile([1, MC], F32, tag="sq1")

                # matmul phase: Ah = relu(W^T A + b), sq += ones^T Ah^2
                for n in range(HT):
                    wcol = wpool.tile([128, HT * 128], F32R, tag="w")
                    nc.sync.dma_start(
                        wcol[:].rearrange("p (t c) -> p t c", c=128),
                        wh[li, :, n * 128 : (n + 1) * 128].rearrange(
                            "(t p) c -> p t c", p=128
                        ),
                    )
                    ps0 = ps_pool.tile([128, MC], F32, tag="ps0")
                    ps1 = ps_pool.tile([128, MC], F32, tag="ps1")
                    for k in range(HT):
                        lhsT = mm_cast(wcol[:, k * 128 : (k + 1) * 128])
                        a_k = A[:, k * NLOC : (k + 1) * NLOC]
                        nc.tensor.matmul(
                            ps0[:], lhsT, mm_cast(a_k[:, 0:MC]),
                            start=(k == 0), stop=(k == HT - 1),
                        )
                        nc.tensor.matmul(
                            ps1[:], lhsT, mm_cast(a_k[:, MC:NLOC]),
                            start=(k == 0), stop=(k == HT - 1),
                        )
                    ah_t = ahw_pool.tile([128, NLOC], F32, tag="ahw")
                    nc.scalar.activation(
                        ah_t[:, 0:MC], ps0[:], ACTF.Relu,
                        bias=bh_t[:, n : n + 1], accum_out=suma_a[:, n : n + 1],
                    )
                    nc.scalar.activation(
                        ah_t[:, MC:NLOC], ps1[:], ACTF.Relu,
                        bias=bh_t[:, n : n + 1], accum_out=suma_b[:, n : n + 1],
                    )
                    ah2_0 = sc_pool.tile([128, MC], F32R, tag="ah2")
                    nc.scalar.activation(ah2_0[:], ah_t[:, 0:MC], ACTF.Square)
                    nc.tensor.matmul(
                        sqp0[:], mm_cast(ones_t[:]), mm_cast(ah2_0[:]),
                        start=(n == 0), stop=(n == HT - 1),
                    )
                    ah2_1 = sc_pool.tile([128, MC], F32R, tag="ah2")
                    nc.scalar.activation(ah2_1[:], ah_t[:, MC:NLOC], ACTF.Square)
                    nc.tensor.matmul(
                        sqp1[:], mm_cast(ones_t[:]), mm_cast(ah2_1[:]),
                        start=(n == 0), stop=(n == HT - 1),
                    )
                    # fold the residual add into the matmul phase (DVE is idle
                    # here); spill Y_partial = Ah + A in bf16 (only feeds the
                    # residual path; sq/M2 stay on the f32 chain)
                    yp_t = ypw_pool.tile([128, NLOC], BF16, tag="ypw")
                    nc.vector.tensor_tensor(
                        yp_t[:], ah_t[:], A[:, n * NLOC : (n + 1) * NLOC], op=ALU.add
                    )
                    nc.sync.dma_start(ah_spill[n * 128 : (n + 1) * 128, :], yp_t[:])

                # local stats: mu_s = (sum_m Ah + sum_m A + sum_m sq) / NLOC
                nc.vector.tensor_copy(sq_sb[:, 0:MC], sqp0[:])
                nc.vector.tensor_copy(sq_sb[:, MC:NLOC], sqp1[:])
                ssq = st2_pool.tile([1, 1], F32, tag="ssq")
                nc.vector.reduce_sum(ssq[:], sq_sb[:], axis=AX)
                nc.gpsimd.partition_broadcast(ssq_bc[:], ssq[:])
                nc.gpsimd.partition_broadcast(bsq[:], sq_sb[:])
                nc.vector.tensor_tensor(tmp1[:], suma_a[:], suma_b[:], op=ALU.add)
                nc.vector.tensor_tensor(tmp1[:], tmp1[:], sumA[:], op=ALU.add)
                nc.vector.tensor_scalar(
                    mu_s[:], tmp1[:], scalar1=ssq_bc[:, 0:1], scalar2=1.0 / NLOC,
                    op0=ALU.add, op1=ALU.mult,
                )
                nc.vector.tensor_tensor(mu2[:], mu_s[:], mu_s[:], op=ALU.mult)

                # W pass: A <- (A - mu_s) + Ah + bsq  (centered pre-BN), M2 accum
                for n in range(HT):
                    a_sl = A[:, n * NLOC : (n + 1) * NLOC]
                    ahr_t = ahr_pool.tile([128, NLOC], BF16, tag="ahr")
                    nc.sync.dma_start(ahr_t[:], ah_spill[n * 128 : (n + 1) * 128, :])
                    nc.vector.scalar_tensor_tensor(
                        a_sl, bsq[:], mu_s[:, n : n + 1], ahr_t[:],
                        op0=ALU.subtract, op1=ALU.add,
                    )
                    sc0 = sqw_pool.tile([128, NLOC], F32, tag="sqw")
                    nc.scalar.activation(
                        sc0[:], a_sl, ACTF.Square,
                        accum_out=m2a[:, n : n + 1],
                    )

                # Welford all-reduce of (mu_s, mu_s^2, M2_s)
                cc_in = dpool.tile([128, 3 * HT], F32, tag="cc_in")
                cc_out = dpool.tile([128, 3 * HT], F32, tag="cc_out")
                nc.gpsimd.dma_start(cc_in[:, 0:HT], mu_s[:])
                nc.gpsimd.dma_start(cc_in[:, HT : 2 * HT], mu2[:])
                nc.gpsimd.dma_start(cc_in[:, 2 * HT : 3 * HT], m2a[:])
                nc.gpsimd.collective_compute(
                    "AllReduce", ALU.add, replica_groups=groups,
                    ins=[cc_in.opt()], outs=[cc_out.opt()],
                )
                red = st2_pool.tile([128, 3 * HT], F32, tag="red")
                nc.gpsimd.dma_start(red[:], cc_out[:])

                # mu = sum(mu_s)/8 ; var = sumM2/N + sum(mu_s^2)/8 - mu^2
                mu = tmp1
                nc.vector.tensor_scalar(
                    mu[:], red[:, 0:HT], scalar1=1.0 / N_CORES, scalar2=None,
                    op0=ALU.mult,
                )
                nc.vector.tensor_scalar(
                    var_t[:], red[:, 2 * HT : 3 * HT], scalar1=1.0 / N, scalar2=None,
                    op0=ALU.mult,
                )
                nc.vector.tensor_scalar(
                    tmp2[:], red[:, HT : 2 * HT], scalar1=1.0 / N_CORES, scalar2=None,
                    op0=ALU.mult,
                )
                nc.vector.tensor_tensor(var_t[:], var_t[:], tmp2[:], op=ALU.add)
                nc.vector.tensor_tensor(tmp2[:], mu[:], mu[:], op=ALU.mult)
                nc.vector.tensor_tensor(var_t[:], var_t[:], tmp2[:], op=ALU.subtract)
                nc.scalar.activation(std_t[:], var_t[:], ACTF.Sqrt, bias=eps_t[:, 0:1])
                nc.vector.reciprocal(scale_t[:], std_t[:])
                # delta = mu - mu_s ; ds = -delta*scale ; A <- A*scale + ds
                nc.vector.tensor_tensor(tmp2[:], mu[:], mu_s[:], op=ALU.subtract)
                nc.vector.tensor_tensor(tmp2[:], tmp2[:], scale_t[:], op=ALU.mult)
                nc.vector.tensor_scalar(
                    ds_t[:], tmp2[:], scalar1=-1.0, scalar2=None, op0=ALU.mult,
                )
                for n in range(HT):
                    a_sl = A[:, n * NLOC : (n + 1) * NLOC]
                    nc.vector.tensor_scalar(
                        a_sl, a_sl, scalar1=scale_t[:, n : n + 1],
                        scalar2=ds_t[:, n : n + 1], op0=ALU.mult, op1=ALU.add,
                    )
                # sum_m of new A per unit = NLOC * ds  (sum of centered W is 0)
                nc.vector.tensor_scalar(
                    sumA[:], ds_t[:], scalar1=float(NLOC), scalar2=None, op0=ALU.mult,
                )

            # ---------------- last layer + final BN ----------------
            wl_t = st_pool.tile([128, HT * KIO], F32R)
            nc.sync.dma_start(
                wl_t[:].rearrange("p (t c) -> p t c", c=KIO),
                wl.rearrange("(t p) c -> p t c", p=128),
            )
            bl_t = st_pool.tile([KIO, 1], F32)
            nc.sync.dma_start(bl_t[:], bl[:].unsqueeze(1))
            psl0 = ps_pool.tile([KIO, MC], F32, tag="ps0")
            psl1 = ps_pool.tile([KIO, MC], F32, tag="ps1")
            for k in range(HT):
                lhsT = mm_cast(wl_t[:, k * KIO : (k + 1) * KIO])
                a_k = A[:, k * NLOC : (k + 1) * NLOC]
                nc.tensor.matmul(
                    psl0[:], lhsT, mm_cast(a_k[:, 0:MC]),
                    start=(k == 0), stop=(k == HT - 1),
                )
                nc.tensor.matmul(
                    psl1[:], lhsT, mm_cast(a_k[:, MC:NLOC]),
                    start=(k == 0), stop=(k == HT - 1),
                )
            yl = st_pool.tile([KIO, NLOC], F32)
            nc.vector.tensor_tensor(yl[:, 0:MC], psl0[:], xtr[:, 0:MC], op=ALU.add)
            nc.vector.tensor_tensor(yl[:, MC:NLOC], psl1[:], xtr[:, MC:NLOC], op=ALU.add)
            nc.vector.tensor_scalar(
                yl[:], yl[:], scalar1=bl_t[:, 0:1], scalar2=None, op0=ALU.add,
            )
            mu_sl = st_pool.tile([KIO, 1], F32)
            nc.vector.reduce_sum(mu_sl[:], yl[:], axis=AX)
            nc.vector.tensor_scalar(
                mu_sl[:], mu_sl[:], scalar1=1.0 / NLOC, scalar2=None, op0=ALU.mult,
            )
            nc.vector.tensor_scalar(
                yl[:], yl[:], scalar1=mu_sl[:, 0:1], scalar2=None, op0=ALU.subtract,
            )
            m2l = st_pool.tile([KIO, 1], F32)
            scr = sqw_pool.tile([KIO, NLOC], F32, tag="sqw")
            nc.scalar.activation(scr[:], yl[:], ACTF.Square, accum_out=m2l[:, 0:1])
            mu2l = st_pool.tile([KIO, 1], F32)
            nc.vector.tensor_tensor(mu2l[:], mu_sl[:], mu_sl[:], op=ALU.mult)
            cpl = st_pool.tile([KIO, 3], F32)
            nc.vector.tensor_copy(cpl[:, 0:1], mu_sl[:])
            nc.vector.tensor_copy(cpl[:, 1:2], mu2l[:])
            nc.vector.tensor_copy(cpl[:, 2:3], m2l[:])
            ccl_in = dpool.tile([KIO, 3], F32, tag="ccl_in")
            ccl_out = dpool.tile([KIO, 3], F32, tag="ccl_out")
            nc.gpsimd.dma_start(ccl_in[:], cpl[:])
            nc.gpsimd.collective_compute(
                "AllReduce", ALU.add, replica_groups=groups,
                ins=[ccl_in.opt()], outs=[ccl_out.opt()],
            )
            redl = st_pool.tile([KIO, 3], F32)
            nc.gpsimd.dma_start(redl[:], ccl_out[:])
            mul_t = st_pool.tile([KIO, 1], F32)
            nc.vector.tensor_scalar(
                mul_t[:], redl[:, 0:1], scalar1=1.0 / N_CORES, scalar2=None,
                op0=ALU.mult,
            )
            varl = st_pool.tile([KIO, 1], F32)
            tl2 = st_pool.tile([KIO, 1], F32)
            nc.vector.tensor_scalar(
                varl[:], redl[:, 2:3], scalar1=1.0 / N, scalar2=None, op0=ALU.mult,
            )
            nc.vector.tensor_scalar(
                tl2[:], redl[:, 1:2], scalar1=1.0 / N_CORES, scalar2=None, op0=ALU.mult,
            )
            nc.vector.tensor_tensor(varl[:], varl[:], tl2[:], op=ALU.add)
            nc.vector.tensor_tensor(tl2[:], mul_t[:], mul_t[:], op=ALU.mult)
            nc.vector.tensor_tensor(varl[:], varl[:], tl2[:], op=ALU.subtract)
            stdl = st_pool.tile([KIO, 1], F32)
            nc.scalar.activation(stdl[:], varl[:], ACTF.Sqrt, bias=eps_t[0:KIO, 0:1])
            scalel = st_pool.tile([KIO, 1], F32)
            nc.vector.reciprocal(scalel[:], stdl[:])
            nc.vector.tensor_tensor(tl2[:], mul_t[:], mu_sl[:], op=ALU.subtract)
            nc.vector.tensor_tensor(tl2[:], tl2[:], scalel[:], op=ALU.mult)
            dsl = st_pool.tile([KIO, 1], F32)
            nc.vector.tensor_scalar(
                dsl[:], tl2[:], scalar1=-1.0, scalar2=None, op0=ALU.mult,
            )
            nc.vector.tensor_scalar(
                yl[:], yl[:], scalar1=scalel[:, 0:1], scalar2=dsl[:, 0:1],
                op0=ALU.mult, op1=ALU.add,
            )
            nc.sync.dma_start(yx.rearrange("m k -> k m"), yl[:])

    nc.compile()
    return nc


def _get_nc():
    if "nc" not in _CACHE:
        _CACHE["nc"] = _build()
    return _CACHE["nc"]


def kernel(**inputs):
    from concourse.bass_utils import run_bass_kernel_spmd

    nc = _get_nc()
    x_in = np.ascontiguousarray(np.asarray(inputs["X_in"], dtype=np.float32))
    shared = {
        "w_first": np.ascontiguousarray(np.asarray(inputs["W_first"], np.float32)),
        "b_first": np.ascontiguousarray(np.asarray(inputs["b_first"], np.float32)),
        "w_h": np.ascontiguousarray(np.asarray(inputs["W_h"], np.float32)),
        "b_h": np.ascontiguousarray(np.asarray(inputs["b_h"], np.float32)),
        "w_last": np.ascontiguousarray(np.asarray(inputs["W_last"], np.float32)),
        "b_last": np.ascontiguousarray(np.asarray(inputs["b_last"], np.float32)),
    }
    in_maps = [
        {"x_in": x_in[c * NLOC : (c + 1) * NLOC], **shared} for c in range(N_CORES)
    ]
    res = run_bass_kernel_spmd(nc, in_maps, list(range(N_CORES)))
    out = np.concatenate([res.results[c]["y"] for c in range(N_CORES)], axis=0)
    return out.astype(np.float32)



# revision 3
# speedup vs baseline: 1.6972x; 1.6972x over previous
"""Trainium2 Bass kernel for nn_DeepNet (dense MLP with BatchNorm over batch).

Reference computation (N=8192 rows, K=2 in/out features, H=4096 hidden, D=3):
    X = relu(X_in @ W_first + b_first)                      # [N, H]
    for i in range(3):
        Xh = relu(X @ W_h[i] + b_h[i])                      # [N, H]
        sq = rowwise_sum(Xh * Xh)                           # [N, 1]
        X  = bn(sq + Xh + X)        # batch stats over N, per hidden unit
    out = bn(X @ W_last + b_last + X_in)                    # [N, 2]

Strategy: data-parallel over N across 8 NeuronCores (1024 rows/core).
Activations live in SBUF transposed: A[h, m] = X[m, h].

The three 4096x4096 matmuls run in fp8 e4m3 with perf_mode=DoubleRow
(2 fp8 values packed per PE cell -> 2x column throughput, 256-deep
contraction per pass).  Weights are pre-quantized on the HOST, scaled by
64 so sigma(W)~1 sits in e4m3's sweet spot; the 64x is carried through
the whole layer (relu is positively homogeneous; BN is scale-invariant)
so no per-element compensation is needed anywhere:
    Ah' = relu(ps + 64 b) = 64 Ah;  sq' = (1/64) ones^T Ah'^2 = 64 sq;
    Y'  = sq' + Ah' + 64 A8;        bn(Y') == bn(Y)  (eps -> 4096 eps).
Activations are stored ONLY in fp8 (A8, unit scale: BN output is
~N(0,1) per unit) -- numpy simulation of this exact quantization gives
rel err 1.1e-2 vs the 2e-2 gate.  The fp32 activation buffer and its
DRAM spill from the fp32r baseline are gone entirely.

BN per layer, built to keep the PE busy:
  - matmul phase also emits YC[n] = Ah' + 64*A8[n] (bf16, SBUF) and
    per-unit sums via activation accum.
  - W pass (DVE+scalar, ~30us): YC[n] += bsq' - mu_local  (centered,
    so bf16 holds the deviation signal), Square-accum -> M2.
  - one 48KB AllReduce of (mu_l, mu_l^2, M2) merges shard stats.
  - scale pass is a SINGLE DVE op per tile writing fp8 A8 directly:
    A8[n] = (YC[n] - (mu_g - mu_l)) * s.  The next layer's matmuls
    depend on A8 tiles individually, so the PE restarts after ~2 tiles.
  - sum_m A8 for the next layer's mean is analytic: NLOC*s*(mu_l-mu_g).
"""

import numpy as np

N_CORES = 8
N = 8192
NLOC = N // N_CORES  # 1024 rows per core
KIO = 2
H = 4096
HT = H // 128  # 32 hidden-dim tiles
K8 = 16  # fp8 DoubleRow contraction tiles (256 logical rows each)
D = 3
MC = 512  # matmul moving-operand chunk
EPS = 1e-5
WS = 64.0  # host-side weight scale for e4m3

_CACHE = {}


def _build():
    import concourse.bass as bass  # noqa: F401  (registers engines)
    import concourse.mybir as mybir
    import concourse.tile as tile
    from concourse import bacc

    F32 = mybir.dt.float32
    F32R = mybir.dt.float32r
    BF16 = mybir.dt.bfloat16
    F8 = mybir.dt.float8e4
    ALU = mybir.AluOpType
    ACTF = mybir.ActivationFunctionType
    AX = mybir.AxisListType.X
    DR = mybir.MatmulPerfMode.DoubleRow

    nc = bacc.Bacc("TRN2", target_bir_lowering=False, debug=False, num_devices=N_CORES)

    xin = nc.dram_tensor("x_in", [NLOC, KIO], F32R, kind="ExternalInput")
    wf = nc.dram_tensor("w_first", [KIO, H], F32R, kind="ExternalInput")
    bf = nc.dram_tensor("b_first", [H], F32, kind="ExternalInput")
    w8 = nc.dram_tensor("w8", [D, HT, 128, K8, 2, 128], F8, kind="ExternalInput")
    bh64 = nc.dram_tensor("bh64", [D, H], F32, kind="ExternalInput")
    wl8 = nc.dram_tensor("wl8", [128, K8, 2, KIO], F8, kind="ExternalInput")
    bl = nc.dram_tensor("b_last", [KIO], F32, kind="ExternalInput")
    yx = nc.dram_tensor("y", [NLOC, KIO], F32, kind="ExternalOutput")

    groups = [list(range(N_CORES))]

    def a8off(n):
        # h-block n lives at pair slot (k8=n//2, j=n%2) of the fp8 buffer
        return ((n >> 1) * 2 + (n & 1)) * NLOC

    with tile.TileContext(nc) as tc:
        with (
            tc.tile_pool(name="big", bufs=1) as big_pool,
            tc.tile_pool(name="w", bufs=2) as wpool,
            tc.tile_pool(name="ah", bufs=2) as ah_pool,
            tc.tile_pool(name="sc", bufs=3) as sc_pool,
            tc.tile_pool(name="sqw", bufs=2) as sqw_pool,
            tc.tile_pool(name="st", bufs=1) as st_pool,
            tc.tile_pool(name="st2", bufs=2) as st2_pool,
            tc.tile_pool(name="ps", bufs=2, space="PSUM") as ps_pool,
            tc.tile_pool(name="sqps", bufs=1, space="PSUM") as sq_pool,
            tc.tile_pool(name="dram", bufs=1, space="DRAM") as dpool,
        ):
            A8 = big_pool.tile([128, K8 * 2 * NLOC], F8)
            YC = big_pool.tile([128, HT * NLOC], BF16)

            ones_f = st_pool.tile([128, 1], F32)
            nc.vector.memset(ones_f[:], 1.0 / WS)
            ones_t = st_pool.tile([128, 1], F32R)
            nc.vector.tensor_copy(ones_t[:], ones_f[:])
            eps_t = st_pool.tile([128, 1], F32)
            nc.vector.memset(eps_t[:], WS * WS * EPS)
            epsl_t = st_pool.tile([KIO, 1], F32)
            nc.vector.memset(epsl_t[:], EPS)
            xtr = st_pool.tile([KIO, NLOC], F32R)
            nc.sync.dma_start(xtr[:], xin.rearrange("m k -> k m"))
            bf_t = st_pool.tile([128, HT], F32)
            nc.sync.dma_start(bf_t[:], bf.rearrange("(t p) -> p t", p=128))

            suma_a = st_pool.tile([128, HT], F32)
            suma_b = st_pool.tile([128, HT], F32)
            sumA64 = st_pool.tile([128, HT], F32)
            m2a = st_pool.tile([128, HT], F32)
            mu_s = st_pool.tile([128, HT], F32)
            mu2 = st_pool.tile([128, HT], F32)
            tmp1 = st_pool.tile([128, HT], F32)
            tmp2 = st_pool.tile([128, HT], F32)
            var_t = st_pool.tile([128, HT], F32)
            std_t = st_pool.tile([128, HT], F32)
            scale_t = st_pool.tile([128, HT], F32)
            dmu_t = st_pool.tile([128, HT], F32)
            sq_sb = st_pool.tile([1, NLOC], F32)
            ssq_bc = st_pool.tile([128, 1], F32)
            bsq = st_pool.tile([128, NLOC], F32)

            # ---------------- first layer: A8 = relu(W_first^T X_in^T + b) --
            wf_t = st_pool.tile([KIO, H], F32R)
            nc.sync.dma_start(wf_t[:], wf[:, :])
            for n in range(HT):
                ps0 = ps_pool.tile([128, MC], F32, tag="ps0")
                ps1 = ps_pool.tile([128, MC], F32, tag="ps1")
                lhsT = wf_t[:, n * 128 : (n + 1) * 128]
                nc.tensor.matmul(ps0[:], lhsT, xtr[:, 0:MC], start=True, stop=True)
                nc.tensor.matmul(ps1[:], lhsT, xtr[:, MC:NLOC], start=True, stop=True)
                base = a8off(n)
                nc.scalar.activation(
                    A8[:, base : base + MC], ps0[:], ACTF.Relu,
                    bias=bf_t[:, n : n + 1], accum_out=suma_a[:, n : n + 1],
                )
                nc.scalar.activation(
                    A8[:, base + MC : base + NLOC], ps1[:], ACTF.Relu,
                    bias=bf_t[:, n : n + 1], accum_out=suma_b[:, n : n + 1],
                )
            # sum_m of the 64-scale residual entering layer 0's Y
            nc.vector.tensor_tensor(tmp1[:], suma_a[:], suma_b[:], op=ALU.add)
            nc.vector.tensor_scalar(
                sumA64[:], tmp1[:], scalar1=WS, scalar2=None, op0=ALU.mult,
            )

            # warm up the collective rings while the PE is busy with layer 0
            ccw_in = dpool.tile([1, 1], F32, tag="ccw_in")
            ccw_out = dpool.tile([1, 1], F32, tag="ccw_out")
            nc.gpsimd.dma_start(ccw_in[:], ones_f[0:1, 0:1])
            nc.gpsimd.collective_compute(
                "AllReduce", ALU.add, replica_groups=groups,
                ins=[ccw_in.opt()], outs=[ccw_out.opt()],
            )

            # ---------------- hidden layers ----------------
            for li in range(D):
                bh_t = st2_pool.tile([128, HT], F32, tag="bh")
                nc.sync.dma_start(bh_t[:], bh64[li].rearrange("(t p) -> p t", p=128))
                sqp0 = sq_pool.tile([1, MC], F32, tag="sq0")
                sqp1 = sq_pool.tile([1, MC], F32, tag="sq1")

                # matmul phase: Ah' = relu(W8^T A8 + 64b), sq' += (1/64) 1^T Ah'^2
                for n in range(HT):
                    wcol = wpool.tile([128, K8 * 2 * 128], F8, tag="w")
                    nc.sync.dma_start(
                        wcol[:].rearrange("p (k j c) -> p k j c", j=2, c=128),
                        w8[li, n],
                    )
                    ps0 = ps_pool.tile([128, MC], F32, tag="ps0")
                    ps1 = ps_pool.tile([128, MC], F32, tag="ps1")
                    for k8 in range(K8):
                        lhsT = wcol[:, k8 * 256 : (k8 + 1) * 256].rearrange(
                            "p (j c) -> p j c", j=2
                        )
                        rhs = A8[:, k8 * 2 * NLOC : (k8 + 1) * 2 * NLOC].rearrange(
                            "p (j m) -> p j m", j=2
                        )
                        nc.tensor.matmul(
                            ps0[:], lhsT, rhs[:, :, 0:MC],
                            start=(k8 == 0), stop=(k8 == K8 - 1), perf_mode=DR,
                        )
                        nc.tensor.matmul(
                            ps1[:], lhsT, rhs[:, :, MC:NLOC],
                            start=(k8 == 0), stop=(k8 == K8 - 1), perf_mode=DR,
                        )
                    ah_t = ah_pool.tile([128, NLOC], F32, tag="ah")
                    nc.scalar.activation(
                        ah_t[:, 0:MC], ps0[:], ACTF.Relu,
                        bias=bh_t[:, n : n + 1], accum_out=suma_a[:, n : n + 1],
                    )
                    nc.scalar.activation(
                        ah_t[:, MC:NLOC], ps1[:], ACTF.Relu,
                        bias=bh_t[:, n : n + 1], accum_out=suma_b[:, n : n + 1],
                    )
                    ah2_0 = sc_pool.tile([128, MC], F32R, tag="ah2")
                    nc.scalar.activation(ah2_0[:], ah_t[:, 0:MC], ACTF.Square)
                    nc.tensor.matmul(
                        sqp0[:], ones_t[:], ah2_0[:],
                        start=(n == 0), stop=(n == HT - 1),
                    )
                    ah2_1 = sc_pool.tile([128, MC], F32R, tag="ah2")
                    nc.scalar.activation(ah2_1[:], ah_t[:, MC:NLOC], ACTF.Square)
                    nc.tensor.matmul(
                        sqp1[:], ones_t[:], ah2_1[:],
                        start=(n == 0), stop=(n == HT - 1),
                    )
                    # YC[n] = Ah' + 64*A8[n]  (DVE is idle during the phase)
                    base = a8off(n)
                    nc.vector.scalar_tensor_tensor(
                        YC[:, n * NLOC : (n + 1) * NLOC],
                        A8[:, base : base + NLOC], WS, ah_t[:],
                        op0=ALU.mult, op1=ALU.add,
                    )

                # stats head: sq' row, its sum, broadcasts, local means
                nc.vector.tensor_copy(sq_sb[:, 0:MC], sqp0[:])
                nc.vector.tensor_copy(sq_sb[:, MC:NLOC], sqp1[:])
                ssq = st2_pool.tile([1, 1], F32, tag="ssq")
                nc.vector.reduce_sum(ssq[:], sq_sb[:], axis=AX)
                nc.gpsimd.partition_broadcast(ssq_bc[:], ssq[:])
                nc.gpsimd.partition_broadcast(bsq[:], sq_sb[:])
                nc.vector.tensor_tensor(tmp1[:], suma_a[:], suma_b[:], op=ALU.add)
                nc.vector.tensor_tensor(tmp1[:], tmp1[:], sumA64[:], op=ALU.add)
                nc.vector.tensor_scalar(
                    mu_s[:], tmp1[:], scalar1=ssq_bc[:, 0:1], scalar2=1.0 / NLOC,
                    op0=ALU.add, op1=ALU.mult,
                )
                nc.vector.tensor_tensor(mu2[:], mu_s[:], mu_s[:], op=ALU.mult)

                # W pass: YC[n] <- YC[n] + bsq' - mu_l  (centered), M2 accum
                for n in range(HT):
                    yc_n = YC[:, n * NLOC : (n + 1) * NLOC]
                    nc.vector.scalar_tensor_tensor(
                        yc_n, bsq[:], mu_s[:, n : n + 1], yc_n,
                        op0=ALU.subtract, op1=ALU.add,
                    )
                    scr = sqw_pool.tile([128, NLOC], BF16, tag="sqw")
                    nc.scalar.activation(
                        scr[:], yc_n, ACTF.Square, accum_out=m2a[:, n : n + 1],
                    )

                # Welford all-reduce of (mu_l, mu_l^2, M2)
                cc_in = dpool.tile([128, 3 * HT], F32, tag="cc_in")
                cc_out = dpool.tile([128, 3 * HT], F32, tag="cc_out")
                nc.gpsimd.dma_start(cc_in[:, 0:HT], mu_s[:])
                nc.gpsimd.dma_start(cc_in[:, HT : 2 * HT], mu2[:])
                nc.gpsimd.dma_start(cc_in[:, 2 * HT : 3 * HT], m2a[:])
                nc.gpsimd.collective_compute(
                    "AllReduce", ALU.add, replica_groups=groups,
                    ins=[cc_in.opt()], outs=[cc_out.opt()],
                )
                red = st2_pool.tile([128, 3 * HT], F32, tag="red")
                nc.gpsimd.dma_start(red[:], cc_out[:])

                # mu = sum(mu_l)/8 ; var = sumM2/N + sum(mu_l^2)/8 - mu^2
                mu = tmp1
                nc.vector.tensor_scalar(
                    mu[:], red[:, 0:HT], scalar1=1.0 / N_CORES, scalar2=None,
                    op0=ALU.mult,
                )
                nc.vector.tensor_scalar(
                    var_t[:], red[:, 2 * HT : 3 * HT], scalar1=1.0 / N, scalar2=None,
                    op0=ALU.mult,
                )
                nc.vector.tensor_scalar(
                    tmp2[:], red[:, HT : 2 * HT], scalar1=1.0 / N_CORES, scalar2=None,
                    op0=ALU.mult,
                )
                nc.vector.tensor_tensor(var_t[:], var_t[:], tmp2[:], op=ALU.add)
                nc.vector.tensor_tensor(tmp2[:], mu[:], mu[:], op=ALU.mult)
                nc.vector.tensor_tensor(var_t[:], var_t[:], tmp2[:], op=ALU.subtract)
                nc.scalar.activation(std_t[:], var_t[:], ACTF.Sqrt, bias=eps_t[:, 0:1])
                nc.vector.reciprocal(scale_t[:], std_t[:])
                # dmu = mu_g - mu_l ; A8[n] = (YC[n] - dmu) * s  (fp8 out)
                nc.vector.tensor_tensor(dmu_t[:], mu[:], mu_s[:], op=ALU.subtract)
                # sum_m of next layer's 64*A8 residual: 64*NLOC*s*(mu_l-mu_g)
                nc.vector.tensor_tensor(tmp2[:], dmu_t[:], scale_t[:], op=ALU.mult)
                nc.vector.tensor_scalar(
                    sumA64[:], tmp2[:], scalar1=-WS * NLOC, scalar2=None, op0=ALU.mult,
                )
                for n in range(HT):
                    base = a8off(n)
                    nc.vector.tensor_scalar(
                        A8[:, base : base + NLOC],
                        YC[:, n * NLOC : (n + 1) * NLOC],
                        scalar1=dmu_t[:, n : n + 1], scalar2=scale_t[:, n : n + 1],
                        op0=ALU.subtract, op1=ALU.mult,
                    )

            # ---------------- last layer + final BN ----------------
            wl_t = st_pool.tile([128, K8 * 2 * KIO], F8)
            nc.sync.dma_start(
                wl_t[:].rearrange("p (k j c) -> p k j c", j=2, c=KIO), wl8[:]
            )
            bl_t = st_pool.tile([KIO, 1], F32)
            nc.sync.dma_start(bl_t[:], bl[:].unsqueeze(1))
            psl0 = ps_pool.tile([KIO, MC], F32, tag="ps0")
            psl1 = ps_pool.tile([KIO, MC], F32, tag="ps1")
            # plain fp8 matmuls: DoubleRow needs pair-stride%16==0, but wl's
            # stationary is only 2 cols wide (stride 2B) -- and it's tiny anyway
            for kk in range(HT):
                lhsT = wl_t[:, kk * KIO : (kk + 1) * KIO]
                base = a8off(kk)
                nc.tensor.matmul(
                    psl0[:], lhsT, A8[:, base : base + MC],
                    start=(kk == 0), stop=(kk == HT - 1),
                )
                nc.tensor.matmul(
                    psl1[:], lhsT, A8[:, base + MC : base + NLOC],
                    start=(kk == 0), stop=(kk == HT - 1),
                )
            yl = st_pool.tile([KIO, NLOC], F32)
            nc.vector.tensor_scalar(
                yl[:, 0:MC], psl0[:], scalar1=1.0 / WS, scalar2=bl_t[:, 0:1],
                op0=ALU.mult, op1=ALU.add,
            )
            nc.vector.tensor_scalar(
                yl[:, MC:NLOC], psl1[:], scalar1=1.0 / WS, scalar2=bl_t[:, 0:1],
                op0=ALU.mult, op1=ALU.add,
            )
            nc.vector.tensor_tensor(yl[:], yl[:], xtr[:], op=ALU.add)
            mu_sl = st_pool.tile([KIO, 1], F32)
            nc.vector.reduce_sum(mu_sl[:], yl[:], axis=AX)
            nc.vector.tensor_scalar(
                mu_sl[:], mu_sl[:], scalar1=1.0 / NLOC, scalar2=None, op0=ALU.mult,
            )
            nc.vector.tensor_scalar(
                yl[:], yl[:], scalar1=mu_sl[:, 0:1], scalar2=None, op0=ALU.subtract,
            )
            m2l = st_pool.tile([KIO, 1], F32)
            scr = sqw_pool.tile([KIO, NLOC], F32, tag="sqw")
            nc.scalar.activation(scr[:], yl[:], ACTF.Square, accum_out=m2l[:, 0:1])
            mu2l = st_pool.tile([KIO, 1], F32)
            nc.vector.tensor_tensor(mu2l[:], mu_sl[:], mu_sl[:], op=ALU.mult)
            cpl = st_pool.tile([KIO, 3], F32)
            nc.vector.tensor_copy(cpl[:, 0:1], mu_sl[:])
            nc.vector.tensor_copy(cpl[:, 1:2], mu2l[:])
            nc.vector.tensor_copy(cpl[:, 2:3], m2l[:])
            ccl_in = dpool.tile([KIO, 3], F32, tag="ccl_in")
            ccl_out = dpool.tile([KIO, 3], F32, tag="ccl_out")
            nc.gpsimd.dma_start(ccl_in[:], cpl[:])
            nc.gpsimd.collective_compute(
                "AllReduce", ALU.add, replica_groups=groups,
                ins=[ccl_in.opt()], outs=[ccl_out.opt()],
            )
            redl = st_pool.tile([KIO, 3], F32)
            nc.gpsimd.dma_start(redl[:], ccl_out[:])
            mul_t = st_pool.tile([KIO, 1], F32)
            nc.vector.tensor_scalar(
                mul_t[:], redl[:, 0:1], scalar1=1.0 / N_CORES, scalar2=None,
                op0=ALU.mult,
            )
            varl = st_pool.tile([KIO, 1], F32)
            tl2 = st_pool.tile([KIO, 1], F32)
            nc.vector.tensor_scalar(
                varl[:], redl[:, 2:3], scalar1=1.0 / N, scalar2=None, op0=ALU.mult,
            )
            nc.vector.tensor_scalar(
                tl2[:], redl[:, 1:2], scalar1=1.0 / N_CORES, scalar2=None, op0=ALU.mult,
            )
            nc.vector.tensor_tensor(varl[:], varl[:], tl2[:], op=ALU.add)
            nc.vector.tensor_tensor(tl2[:], mul_t[:], mul_t[:], op=ALU.mult)
            nc.vector.tensor_tensor(varl[:], varl[:], tl2[:], op=ALU.subtract)
            stdl = st_pool.tile([KIO, 1], F32)
            nc.scalar.activation(stdl[:], varl[:], ACTF.Sqrt, bias=epsl_t[:, 0:1])
            scalel = st_pool.tile([KIO, 1], F32)
            nc.vector.reciprocal(scalel[:], stdl[:])
            nc.vector.tensor_tensor(tl2[:], mul_t[:], mu_sl[:], op=ALU.subtract)
            nc.vector.tensor_tensor(tl2[:], tl2[:], scalel[:], op=ALU.mult)
            dsl = st_pool.tile([KIO, 1], F32)
            nc.vector.tensor_scalar(
                dsl[:], tl2[:], scalar1=-1.0, scalar2=None, op0=ALU.mult,
            )
            nc.vector.tensor_scalar(
                yl[:], yl[:], scalar1=scalel[:, 0:1], scalar2=dsl[:, 0:1],
                op0=ALU.mult, op1=ALU.add,
            )
            nc.sync.dma_start(yx.rearrange("m k -> k m"), yl[:])

    nc.compile()
    return nc


def _get_nc():
    if "nc" not in _CACHE:
        _CACHE["nc"] = _build()
    return _CACHE["nc"]


def _prep_in_maps(inputs):
    import ml_dtypes

    E4 = ml_dtypes.float8_e4m3  # TRN FP8_EXP4 bit-compatible (max 240)
    x_in = np.ascontiguousarray(np.asarray(inputs["X_in"], dtype=np.float32))
    wh = np.asarray(inputs["W_h"], np.float32)
    w8 = (WS * wh).astype(E4)  # [D, 4096, 4096]
    w8 = w8.reshape(D, K8, 2, 128, HT, 128)  # h -> (k8, j, p); out -> (n, c)
    w8 = np.ascontiguousarray(w8.transpose(0, 4, 3, 1, 2, 5))  # [D, n, p, k8, j, c]
    wl = np.asarray(inputs["W_last"], np.float32)
    wl8 = (WS * wl).astype(E4).reshape(K8, 2, 128, KIO)
    wl8 = np.ascontiguousarray(wl8.transpose(2, 0, 1, 3))  # [p, k8, j, c]
    shared = {
        "w_first": np.ascontiguousarray(np.asarray(inputs["W_first"], np.float32)),
        "b_first": np.ascontiguousarray(np.asarray(inputs["b_first"], np.float32)),
        "w8": w8,
        "bh64": np.ascontiguousarray(WS * np.asarray(inputs["b_h"], np.float32)),
        "wl8": wl8,
        "b_last": np.ascontiguousarray(np.asarray(inputs["b_last"], np.float32)),
    }
    return [
        {"x_in": x_in[c * NLOC : (c + 1) * NLOC], **shared} for c in range(N_CORES)
    ]


def kernel(**inputs):
    from concourse.bass_utils import run_bass_kernel_spmd

    nc = _get_nc()
    in_maps = _prep_in_maps(inputs)
    res = run_bass_kernel_spmd(nc, in_maps, list(range(N_CORES)))
    out = np.concatenate([res.results[c]["y"] for c in range(N_CORES)], axis=0)
    return out.astype(np.float32)


# revision 11
# speedup vs baseline: 1.7064x; 1.0054x over previous
"""Trainium2 Bass kernel for nn_DeepNet (dense MLP with BatchNorm over batch).

Reference computation (N=8192 rows, K=2 in/out features, H=4096 hidden, D=3):
    X = relu(X_in @ W_first + b_first)                      # [N, H]
    for i in range(3):
        Xh = relu(X @ W_h[i] + b_h[i])                      # [N, H]
        sq = rowwise_sum(Xh * Xh)                           # [N, 1]
        X  = bn(sq + Xh + X)        # batch stats over N, per hidden unit
    out = bn(X @ W_last + b_last + X_in)                    # [N, 2]

Strategy: data-parallel over N across 8 NeuronCores (1024 rows/core).
Activations live in SBUF transposed: A[h, m] = X[m, h].

The three 4096x4096 matmuls run in fp8 e4m3 with perf_mode=DoubleRow
(2 fp8 values packed per PE cell -> 2x column throughput, 256-deep
contraction per pass).  Weights are pre-quantized on the HOST, scaled by
64 so sigma(W)~1 sits in e4m3's sweet spot; the 64x is carried through
the whole layer (relu is positively homogeneous; BN is scale-invariant)
so no per-element compensation is needed anywhere:
    Ah' = relu(ps + 64 b) = 64 Ah;  sq' = (1/64) ones^T Ah'^2 = 64 sq;
    Y'  = sq' + Ah' + 64 A8;        bn(Y') == bn(Y)  (eps -> 4096 eps).
Activations are stored ONLY in fp8 (A8, unit scale: BN output is
~N(0,1) per unit) -- numpy simulation of this exact quantization gives
rel err 1.1e-2 vs the 2e-2 gate.  The fp32 activation buffer and its
DRAM spill from the fp32r baseline are gone entirely.

BN per layer, built to keep the PE busy:
  - matmul phase also emits YC[n] = Ah' + 64*A8[n] (bf16, SBUF) and
    per-unit sums via activation accum.
  - W pass (DVE+scalar, ~30us): YC[n] += bsq' - mu_local  (centered,
    so bf16 holds the deviation signal), Square-accum -> M2.
  - one 48KB AllReduce of (mu_l, mu_l^2, M2) merges shard stats.
  - scale pass is a SINGLE DVE op per tile writing fp8 A8 directly:
    A8[n] = (YC[n] - (mu_g - mu_l)) * s.  The next layer's matmuls
    depend on A8 tiles individually, so the PE restarts after ~2 tiles.
  - sum_m A8 for the next layer's mean is analytic: NLOC*s*(mu_l-mu_g).
"""

import numpy as np

N_CORES = 8
N = 8192
NLOC = N // N_CORES  # 1024 rows per core
KIO = 2
H = 4096
HT = H // 128  # 32 hidden-dim tiles
K8 = 16  # fp8 DoubleRow contraction tiles (256 logical rows each)
D = 3
MC = 512  # matmul moving-operand chunk
EPS = 1e-5
WS = 64.0  # host-side weight scale for e4m3

_CACHE = {}


def _build():
    import concourse.bass as bass  # noqa: F401  (registers engines)
    import concourse.mybir as mybir
    import concourse.tile as tile
    from concourse import bacc

    F32 = mybir.dt.float32
    F32R = mybir.dt.float32r
    BF16 = mybir.dt.bfloat16
    F8 = mybir.dt.float8e4
    ALU = mybir.AluOpType
    ACTF = mybir.ActivationFunctionType
    AX = mybir.AxisListType.X
    DR = mybir.MatmulPerfMode.DoubleRow

    nc = bacc.Bacc("TRN2", target_bir_lowering=False, debug=False, num_devices=N_CORES)

    xin = nc.dram_tensor("x_in", [NLOC, KIO], F32R, kind="ExternalInput")
    wf = nc.dram_tensor("w_first", [KIO, H], F32R, kind="ExternalInput")
    bf = nc.dram_tensor("b_first", [H], F32, kind="ExternalInput")
    w8 = nc.dram_tensor("w8", [D, HT, 128, K8, 2, 128], F8, kind="ExternalInput")
    bh64 = nc.dram_tensor("bh64", [D, H], F32, kind="ExternalInput")
    wlbf = nc.dram_tensor("wl_bf", [128, HT, KIO], BF16, kind="ExternalInput")
    bl = nc.dram_tensor("b_last", [KIO], F32, kind="ExternalInput")
    yx = nc.dram_tensor("y", [NLOC, KIO], F32, kind="ExternalOutput")

    groups = [list(range(N_CORES))]

    def a8off(n):
        # h-block n lives at pair slot (k8=n//2, j=n%2) of the fp8 buffer
        return ((n >> 1) * 2 + (n & 1)) * NLOC

    with tile.TileContext(nc) as tc:
        with (
            tc.tile_pool(name="big", bufs=1) as big_pool,
            tc.tile_pool(name="w", bufs=2) as wpool,
            tc.tile_pool(name="ah", bufs=2) as ah_pool,
            tc.tile_pool(name="sc", bufs=3) as sc_pool,
            tc.tile_pool(name="sqw", bufs=2) as sqw_pool,
            tc.tile_pool(name="st", bufs=1) as st_pool,
            tc.tile_pool(name="st2", bufs=2) as st2_pool,
            tc.tile_pool(name="ps", bufs=2, space="PSUM") as ps_pool,
            tc.tile_pool(name="sqps", bufs=1, space="PSUM") as sq_pool,
            tc.tile_pool(name="dram", bufs=1, space="DRAM") as dpool,
        ):
            A8 = big_pool.tile([128, K8 * 2 * NLOC], F8)
            YC = big_pool.tile([128, HT * NLOC], BF16)

            ones_f = st_pool.tile([128, 1], F32)
            nc.vector.memset(ones_f[:], 1.0 / WS)
            ones_t = st_pool.tile([128, 1], F32R)
            nc.vector.tensor_copy(ones_t[:], ones_f[:])
            eps_t = st_pool.tile([128, 1], F32)
            nc.vector.memset(eps_t[:], WS * WS * EPS)
            epsl_t = st_pool.tile([KIO, 1], F32)
            nc.vector.memset(epsl_t[:], EPS)
            xtr = st_pool.tile([KIO, NLOC], F32R)
            nc.sync.dma_start(xtr[:], xin.rearrange("m k -> k m"))
            bf_t = st_pool.tile([128, HT], F32)
            nc.sync.dma_start(bf_t[:], bf.rearrange("(t p) -> p t", p=128))

            suma_a = st_pool.tile([128, HT], F32)
            suma_b = st_pool.tile([128, HT], F32)
            sumA64 = st_pool.tile([128, HT], F32)
            m2a = st_pool.tile([128, HT], F32)
            mu_s = st_pool.tile([128, HT], F32)
            mu2 = st_pool.tile([128, HT], F32)
            tmp1 = st_pool.tile([128, HT], F32)
            tmp2 = st_pool.tile([128, HT], F32)
            var_t = st_pool.tile([128, HT], F32)
            std_t = st_pool.tile([128, HT], F32)
            scale_t = st_pool.tile([128, HT], F32)
            dmu_t = st_pool.tile([128, HT], F32)
            sq_sb = st_pool.tile([1, NLOC], F32)
            sq_bf = st_pool.tile([1, NLOC], BF16)
            ssq_bc = st_pool.tile([128, 1], F32)
            bsq = st_pool.tile([128, NLOC], BF16)

            # ---------------- first layer: A8 = relu(W_first^T X_in^T + b) --
            wf_t = st_pool.tile([KIO, H], F32R)
            nc.sync.dma_start(wf_t[:], wf[:, :])
            for n in range(HT):
                ps0 = ps_pool.tile([128, MC], F32, tag="ps0")
                ps1 = ps_pool.tile([128, MC], F32, tag="ps1")
                lhsT = wf_t[:, n * 128 : (n + 1) * 128]
                nc.tensor.matmul(ps0[:], lhsT, xtr[:, 0:MC], start=True, stop=True)
                nc.tensor.matmul(ps1[:], lhsT, xtr[:, MC:NLOC], start=True, stop=True)
                base = a8off(n)
                nc.scalar.activation(
                    A8[:, base : base + MC], ps0[:], ACTF.Relu,
                    bias=bf_t[:, n : n + 1], accum_out=suma_a[:, n : n + 1],
                )
                nc.scalar.activation(
                    A8[:, base + MC : base + NLOC], ps1[:], ACTF.Relu,
                    bias=bf_t[:, n : n + 1], accum_out=suma_b[:, n : n + 1],
                )
            # sum_m of the 64-scale residual entering layer 0's Y
            nc.vector.tensor_tensor(tmp1[:], suma_a[:], suma_b[:], op=ALU.add)
            nc.vector.tensor_scalar(
                sumA64[:], tmp1[:], scalar1=WS, scalar2=None, op0=ALU.mult,
            )

            # last-layer weights: load early, folded with BN scale at the end
            wl_t = st_pool.tile([128, HT * KIO], BF16)
            nc.sync.dma_start(wl_t[:], wlbf[:])
            bl_t = st_pool.tile([KIO, 1], F32)
            nc.sync.dma_start(bl_t[:], bl[:].unsqueeze(1))

            # warm up the collective rings while the PE is busy with layer 0
            ccw_in = dpool.tile([1, 1], F32, tag="ccw_in")
            ccw_out = dpool.tile([1, 1], F32, tag="ccw_out")
            nc.gpsimd.dma_start(ccw_in[:], ones_f[0:1, 0:1])
            nc.gpsimd.collective_compute(
                "AllReduce", ALU.add, replica_groups=groups,
                ins=[ccw_in.opt()], outs=[ccw_out.opt()],
            )

            # ---------------- hidden layers ----------------
            for li in range(D):
                bh_t = st2_pool.tile([128, HT], F32, tag="bh")
                nc.sync.dma_start(bh_t[:], bh64[li].rearrange("(t p) -> p t", p=128))
                sqp0 = sq_pool.tile([1, MC], F32, tag="sq0")
                sqp1 = sq_pool.tile([1, MC], F32, tag="sq1")

                # matmul phase: Ah' = relu(W8^T A8 + 64b), sq' += (1/64) 1^T Ah'^2
                for n in range(HT):
                    wcol = wpool.tile([128, K8 * 2 * 128], F8, tag="w")
                    nc.sync.dma_start(
                        wcol[:].rearrange("p (k j c) -> p k j c", j=2, c=128),
                        w8[li, n],
                    )
                    ps0 = ps_pool.tile([128, MC], F32, tag="ps0")
                    ps1 = ps_pool.tile([128, MC], F32, tag="ps1")
                    for k8 in range(K8):
                        lhsT = wcol[:, k8 * 256 : (k8 + 1) * 256].rearrange(
                            "p (j c) -> p j c", j=2
                        )
                        rhs = A8[:, k8 * 2 * NLOC : (k8 + 1) * 2 * NLOC].rearrange(
                            "p (j m) -> p j m", j=2
                        )
                        nc.tensor.matmul(
                            ps0[:], lhsT, rhs[:, :, 0:MC],
                            start=(k8 == 0), stop=(k8 == K8 - 1), perf_mode=DR,
                        )
                        nc.tensor.matmul(
                            ps1[:], lhsT, rhs[:, :, MC:NLOC],
                            start=(k8 == 0), stop=(k8 == K8 - 1), perf_mode=DR,
                        )
                    ah_t = ah_pool.tile([128, NLOC], BF16, tag="ah")
                    nc.scalar.activation(
                        ah_t[:, 0:MC], ps0[:], ACTF.Relu,
                        bias=bh_t[:, n : n + 1], accum_out=suma_a[:, n : n + 1],
                    )
                    nc.scalar.activation(
                        ah_t[:, MC:NLOC], ps1[:], ACTF.Relu,
                        bias=bh_t[:, n : n + 1], accum_out=suma_b[:, n : n + 1],
                    )
                    ah2_0 = sc_pool.tile([128, MC], F32R, tag="ah2")
                    nc.scalar.activation(ah2_0[:], ah_t[:, 0:MC], ACTF.Square)
                    nc.tensor.matmul(
                        sqp0[:], ones_t[:], ah2_0[:],
                        start=(n == 0), stop=(n == HT - 1),
                    )
                    ah2_1 = sc_pool.tile([128, MC], F32R, tag="ah2")
                    nc.scalar.activation(ah2_1[:], ah_t[:, MC:NLOC], ACTF.Square)
                    nc.tensor.matmul(
                        sqp1[:], ones_t[:], ah2_1[:],
                        start=(n == 0), stop=(n == HT - 1),
                    )
                    # YC[n] = Ah' + 64*A8[n]  (DVE is idle during the phase)
                    base = a8off(n)
                    nc.vector.scalar_tensor_tensor(
                        YC[:, n * NLOC : (n + 1) * NLOC],
                        A8[:, base : base + NLOC], WS, ah_t[:],
                        op0=ALU.mult, op1=ALU.add,
                    )

                # stats head: sq' row, its sum, broadcasts, local means
                nc.vector.tensor_copy(sq_sb[:, 0:MC], sqp0[:])
                nc.vector.tensor_copy(sq_sb[:, MC:NLOC], sqp1[:])
                ssq = st2_pool.tile([1, 1], F32, tag="ssq")
                nc.vector.reduce_sum(ssq[:], sq_sb[:], axis=AX)
                nc.vector.tensor_copy(sq_bf[:], sq_sb[:])
                nc.gpsimd.partition_broadcast(ssq_bc[:], ssq[:])
                nc.gpsimd.partition_broadcast(bsq[:], sq_bf[:])
                nc.vector.tensor_tensor(tmp1[:], suma_a[:], suma_b[:], op=ALU.add)
                nc.vector.tensor_tensor(tmp1[:], tmp1[:], sumA64[:], op=ALU.add)
                nc.vector.tensor_scalar(
                    mu_s[:], tmp1[:], scalar1=ssq_bc[:, 0:1], scalar2=1.0 / NLOC,
                    op0=ALU.add, op1=ALU.mult,
                )
                nc.vector.tensor_tensor(mu2[:], mu_s[:], mu_s[:], op=ALU.mult)

                # W pass: YC[n] <- YC[n] + bsq' - mu_l  (centered), M2 accum
                for n in range(HT):
                    yc_n = YC[:, n * NLOC : (n + 1) * NLOC]
                    nc.vector.scalar_tensor_tensor(
                        yc_n, bsq[:], mu_s[:, n : n + 1], yc_n,
                        op0=ALU.subtract, op1=ALU.add,
                    )
                    scr = sqw_pool.tile([128, NLOC], BF16, tag="sqw")
                    nc.scalar.activation(
                        scr[:], yc_n, ACTF.Square, accum_out=m2a[:, n : n + 1],
                    )

                # Welford all-reduce of (mu_l, mu_l^2, M2)
                cc_in = dpool.tile([128, 3 * HT], F32, tag="cc_in")
                cc_out = dpool.tile([128, 3 * HT], F32, tag="cc_out")
                nc.gpsimd.dma_start(cc_in[:, 0:HT], mu_s[:])
                nc.gpsimd.dma_start(cc_in[:, HT : 2 * HT], mu2[:])
                nc.gpsimd.dma_start(cc_in[:, 2 * HT : 3 * HT], m2a[:])
                nc.gpsimd.collective_compute(
                    "AllReduce", ALU.add, replica_groups=groups,
                    ins=[cc_in.opt()], outs=[cc_out.opt()],
                )
                red = st2_pool.tile([128, 3 * HT], F32, tag="red")
                nc.gpsimd.dma_start(red[:], cc_out[:])

                # mu = sum(mu_l)/8 ; var = sumM2/N + sum(mu_l^2)/8 - mu^2
                mu = tmp1
                nc.vector.tensor_scalar(
                    mu[:], red[:, 0:HT], scalar1=1.0 / N_CORES, scalar2=None,
                    op0=ALU.mult,
                )
                nc.vector.tensor_scalar(
                    var_t[:], red[:, 2 * HT : 3 * HT], scalar1=1.0 / N, scalar2=None,
                    op0=ALU.mult,
                )
                nc.vector.tensor_scalar(
                    tmp2[:], red[:, HT : 2 * HT], scalar1=1.0 / N_CORES, scalar2=None,
                    op0=ALU.mult,
                )
                nc.vector.tensor_tensor(var_t[:], var_t[:], tmp2[:], op=ALU.add)
                nc.vector.tensor_tensor(tmp2[:], mu[:], mu[:], op=ALU.mult)
                nc.vector.tensor_tensor(var_t[:], var_t[:], tmp2[:], op=ALU.subtract)
                nc.scalar.activation(std_t[:], var_t[:], ACTF.Sqrt, bias=eps_t[:, 0:1])
                nc.vector.reciprocal(scale_t[:], std_t[:])
                # dmu = mu_g - mu_l
                nc.vector.tensor_tensor(dmu_t[:], mu[:], mu_s[:], op=ALU.subtract)
                if li < D - 1:
                    # sum_m of next layer's 64*A8 residual: 64*NLOC*s*(mu_l-mu_g)
                    nc.vector.tensor_tensor(tmp2[:], dmu_t[:], scale_t[:], op=ALU.mult)
                    nc.vector.tensor_scalar(
                        sumA64[:], tmp2[:], scalar1=-WS * NLOC, scalar2=None,
                        op0=ALU.mult,
                    )
                    # A8[n] = (YC[n] - dmu) * s  (fp8 out)
                    for n in range(HT):
                        base = a8off(n)
                        nc.vector.tensor_scalar(
                            A8[:, base : base + NLOC],
                            YC[:, n * NLOC : (n + 1) * NLOC],
                            scalar1=dmu_t[:, n : n + 1], scalar2=scale_t[:, n : n + 1],
                            op0=ALU.subtract, op1=ALU.mult,
                        )

            # ---------------- last layer + final BN ----------------
            # X3 = (YC - dmu)*s, so X3 @ W_last = YC @ (s*W_last) - dmu @ (s*W_last)
            # -- run in bf16 straight off the centered YC (no fp8 error here).
            wl_s = st_pool.tile([128, HT * KIO], BF16)
            dmu_bf = st_pool.tile([128, HT], BF16)
            nc.vector.tensor_copy(dmu_bf[:], dmu_t[:])
            for kk in range(HT):
                nc.vector.tensor_scalar(
                    wl_s[:, kk * KIO : (kk + 1) * KIO],
                    wl_t[:, kk * KIO : (kk + 1) * KIO],
                    scalar1=scale_t[:, kk : kk + 1], scalar2=None, op0=ALU.mult,
                )
            corr_ps = sq_pool.tile([KIO, 1], F32, tag="corr")
            psl0 = ps_pool.tile([KIO, MC], F32, tag="ps0")
            psl1 = ps_pool.tile([KIO, MC], F32, tag="ps1")
            for kk in range(HT):
                lhsT = wl_s[:, kk * KIO : (kk + 1) * KIO]
                nc.tensor.matmul(
                    corr_ps[:], lhsT, dmu_bf[:, kk : kk + 1],
                    start=(kk == 0), stop=(kk == HT - 1),
                )
                nc.tensor.matmul(
                    psl0[:], lhsT, YC[:, kk * NLOC : kk * NLOC + MC],
                    start=(kk == 0), stop=(kk == HT - 1),
                )
                nc.tensor.matmul(
                    psl1[:], lhsT, YC[:, kk * NLOC + MC : (kk + 1) * NLOC],
                    start=(kk == 0), stop=(kk == HT - 1),
                )
            corr_sb = st_pool.tile([KIO, 1], F32)
            nc.vector.tensor_copy(corr_sb[:], corr_ps[:])
            yl = st_pool.tile([KIO, NLOC], F32)
            nc.vector.tensor_scalar(
                yl[:, 0:MC], psl0[:], scalar1=corr_sb[:, 0:1], scalar2=bl_t[:, 0:1],
                op0=ALU.subtract, op1=ALU.add,
            )
            nc.vector.tensor_scalar(
                yl[:, MC:NLOC], psl1[:], scalar1=corr_sb[:, 0:1], scalar2=bl_t[:, 0:1],
                op0=ALU.subtract, op1=ALU.add,
            )
            nc.vector.tensor_tensor(yl[:], yl[:], xtr[:], op=ALU.add)
            mu_sl = st_pool.tile([KIO, 1], F32)
            nc.vector.reduce_sum(mu_sl[:], yl[:], axis=AX)
            nc.vector.tensor_scalar(
                mu_sl[:], mu_sl[:], scalar1=1.0 / NLOC, scalar2=None, op0=ALU.mult,
            )
            nc.vector.tensor_scalar(
                yl[:], yl[:], scalar1=mu_sl[:, 0:1], scalar2=None, op0=ALU.subtract,
            )
            m2l = st_pool.tile([KIO, 1], F32)
            scr = sqw_pool.tile([KIO, NLOC], F32, tag="sqw")
            nc.scalar.activation(scr[:], yl[:], ACTF.Square, accum_out=m2l[:, 0:1])
            mu2l = st_pool.tile([KIO, 1], F32)
            nc.vector.tensor_tensor(mu2l[:], mu_sl[:], mu_sl[:], op=ALU.mult)
            cpl = st_pool.tile([KIO, 3], F32)
            nc.vector.tensor_copy(cpl[:, 0:1], mu_sl[:])
            nc.vector.tensor_copy(cpl[:, 1:2], mu2l[:])
            nc.vector.tensor_copy(cpl[:, 2:3], m2l[:])
            ccl_in = dpool.tile([KIO, 3], F32, tag="ccl_in")
            ccl_out = dpool.tile([KIO, 3], F32, tag="ccl_out")
            nc.gpsimd.dma_start(ccl_in[:], cpl[:])
            nc.gpsimd.collective_compute(
                "AllReduce", ALU.add, replica_groups=groups,
                ins=[ccl_in.opt()], outs=[ccl_out.opt()],
            )
            redl = st_pool.tile([KIO, 3], F32)
            nc.gpsimd.dma_start(redl[:], ccl_out[:])
            mul_t = st_pool.tile([KIO, 1], F32)
            nc.vector.tensor_scalar(
                mul_t[:], redl[:, 0:1], scalar1=1.0 / N_CORES, scalar2=None,
                op0=ALU.mult,
            )
            varl = st_pool.tile([KIO, 1], F32)
            tl2 = st_pool.tile([KIO, 1], F32)
            nc.vector.tensor_scalar(
                varl[:], redl[:, 2:3], scalar1=1.0 / N, scalar2=None, op0=ALU.mult,
            )
            nc.vector.tensor_scalar(
                tl2[:], redl[:, 1:2], scalar1=1.0 / N_CORES, scalar2=None, op0=ALU.mult,
            )
            nc.vector.tensor_tensor(varl[:], varl[:], tl2[:], op=ALU.add)
            nc.vector.tensor_tensor(tl2[:], mul_t[:], mul_t[:], op=ALU.mult)
            nc.vector.tensor_tensor(varl[:], varl[:], tl2[:], op=ALU.subtract)
            stdl = st_pool.tile([KIO, 1], F32)
            nc.scalar.activation(stdl[:], varl[:], ACTF.Sqrt, bias=epsl_t[:, 0:1])
            scalel = st_pool.tile([KIO, 1], F32)
            nc.vector.reciprocal(scalel[:], stdl[:])
            nc.vector.tensor_tensor(tl2[:], mul_t[:], mu_sl[:], op=ALU.subtract)
            nc.vector.tensor_tensor(tl2[:], tl2[:], scalel[:], op=ALU.mult)
            dsl = st_pool.tile([KIO, 1], F32)
            nc.vector.tensor_scalar(
                dsl[:], tl2[:], scalar1=-1.0, scalar2=None, op0=ALU.mult,
            )
            nc.vector.tensor_scalar(
                yl[:], yl[:], scalar1=scalel[:, 0:1], scalar2=dsl[:, 0:1],
                op0=ALU.mult, op1=ALU.add,
            )
            nc.sync.dma_start(yx.rearrange("m k -> k m"), yl[:])

    nc.compile()
    return nc


def _get_nc():
    if "nc" not in _CACHE:
        _CACHE["nc"] = _build()
    return _CACHE["nc"]


def _prep_in_maps(inputs):
    import ml_dtypes

    E4 = ml_dtypes.float8_e4m3  # TRN FP8_EXP4 bit-compatible (max 240)
    x_in = np.ascontiguousarray(np.asarray(inputs["X_in"], dtype=np.float32))
    wh = np.asarray(inputs["W_h"], np.float32)
    w8 = (WS * wh).astype(E4)  # [D, 4096, 4096]
    w8 = w8.reshape(D, K8, 2, 128, HT, 128)  # h -> (k8, j, p); out -> (n, c)
    w8 = np.ascontiguousarray(w8.transpose(0, 4, 3, 1, 2, 5))  # [D, n, p, k8, j, c]
    wl = np.asarray(inputs["W_last"], np.float32)
    wl_bf = wl.astype(ml_dtypes.bfloat16).reshape(HT, 128, KIO)
    wl_bf = np.ascontiguousarray(wl_bf.transpose(1, 0, 2))  # [p, t, c]
    shared = {
        "w_first": np.ascontiguousarray(np.asarray(inputs["W_first"], np.float32)),
        "b_first": np.ascontiguousarray(np.asarray(inputs["b_first"], np.float32)),
        "w8": w8,
        "bh64": np.ascontiguousarray(WS * np.asarray(inputs["b_h"], np.float32)),
        "wl_bf": wl_bf,
        "b_last": np.ascontiguousarray(np.asarray(inputs["b_last"], np.float32)),
    }
    return [
        {"x_in": x_in[c * NLOC : (c + 1) * NLOC], **shared} for c in range(N_CORES)
    ]


def kernel(**inputs):
    from concourse.bass_utils import run_bass_kernel_spmd

    nc = _get_nc()
    in_maps = _prep_in_maps(inputs)
    res = run_bass_kernel_spmd(nc, in_maps, list(range(N_CORES)))
    out = np.concatenate([res.results[c]["y"] for c in range(N_CORES)], axis=0)
    return out.astype(np.float32)
